# revision 1
# baseline (speedup 1.0000x reference)
"""Trainium2 Bass kernel for the sparse CG tensor product (CudaTensorProduct).

Math: out[b, o] = sum_k cb[k] * in1[b, i1[k]] * in2[b, i2[k]]  (scatter on io[k])
Rewritten as a dense bilinear contraction:
    z[b, q] = in1[b, q % 36] * in2[b, q // 36]        (q = j*36 + i, 324 rows)
    out[b]  = z[b] @ W2          with W2[j*36+i, o] = sum of cb over (i, j, o)

Per-core mapping (8-way batch data-parallel, 8192 rows/core):
  - batch tiles of 512 rows, laid out [128 partitions, 4 sub-batches]
  - PE transpose -> inT [45, 512] (features on partitions)
  - rep1 (stacked in1T) via SBUF->SBUF DMAs; rep2 (row-replicated in2T)
    via PE selection matmuls; z = rep1 * rep2 on DVE (float32r rounded)
  - out[128, 324] = sum over 3 K-chunks of z_c.T @ W2_c  (float32r matmuls,
    PSUM accumulate), copied to SBUF, DMA'd back batch-major.
"""
import sys
if '/opt/trn_rl_repo' not in sys.path:
    sys.path.insert(0, '/opt/trn_rl_repo')

import numpy as np

N_CORES = 8
B = 65536
BC = B // N_CORES          # 8192 batch rows per core
BT = 512                   # batch rows per pipeline tile
TS = BT // 128             # sub-batches per tile (4)
NT = BC // BT              # tiles per core (16)
D1, D2 = 36, 9
DF = D1 + D2               # 45
DZ = D1 * D2               # 324 z rows
DO = 324                   # out columns
NCH = 3                    # K chunks
KC = DZ // NCH             # 108 rows per chunk

_CACHE: dict = {}


def _build_program():
    import concourse.bass as bass
    import concourse.mybir as mybir
    import concourse.tile as tile
    from concourse import bacc
    from concourse.masks import make_identity

    f32 = mybir.dt.float32
    f32r = mybir.dt.float32r

    nc = bacc.Bacc("TRN2", target_bir_lowering=False, debug=False)
    in1 = nc.dram_tensor("in1", [BC, D1], f32, kind="ExternalInput").ap()
    in2 = nc.dram_tensor("in2", [BC, D2], f32, kind="ExternalInput").ap()
    w = nc.dram_tensor("w2", [NCH, KC, DO], f32r, kind="ExternalInput").ap()
    s2 = nc.dram_tensor("s2", [NCH, DF, KC], f32r, kind="ExternalInput").ap()
    out = nc.dram_tensor("out", [BC, DO], f32, kind="ExternalOutput").ap()

    with tile.TileContext(nc) as tc:
        with (
            tc.tile_pool(name="consts", bufs=1) as consts,
            tc.tile_pool(name="loads", bufs=3) as loads,
            tc.tile_pool(name="xts", bufs=3) as xts,
            tc.tile_pool(name="reps", bufs=3) as repp,
            tc.tile_pool(name="zs", bufs=2) as zp,
            tc.tile_pool(name="obs", bufs=3) as obs,
            tc.tile_pool(name="pst", bufs=2, space="PSUM") as pst,
            tc.tile_pool(name="psr", bufs=2, space="PSUM") as psr,
            tc.tile_pool(name="pso", bufs=3, space="PSUM") as pso,
        ):
            ident = consts.tile([128, 128], f32)
            make_identity(nc, ident)
            w_sb = consts.tile([KC, NCH, DO], f32r)
            nc.sync.dma_start(out=w_sb, in_=w.rearrange("c k n -> k c n"))
            s2_sb = consts.tile([DF, NCH, KC], f32r)
            nc.sync.dma_start(out=s2_sb, in_=s2.rearrange("c k n -> k c n"))

            in1r = in1.rearrange("(t p s) f -> t p s f", p=128, s=TS)
            in2r = in2.rearrange("(t p s) f -> t p s f", p=128, s=TS)
            outr = out.rearrange("(t p s) f -> t p s f", p=128, s=TS)

            for t in range(NT):
                A = loads.tile([128, TS, DF], f32)
                nc.sync.dma_start(out=A[:, :, 0:D1], in_=in1r[t])
                nc.sync.dma_start(out=A[:, :, D1:DF], in_=in2r[t])

                xT = xts.tile([DF, TS, 128], f32r)
                for s in range(TS):
                    pt = pst.tile([DF, 128], f32)
                    nc.tensor.transpose(pt, A[:, s, :], ident)
                    nc.scalar.copy(out=xT[:, s, :], in_=pt)

                rep1 = repp.tile([KC, TS, 128], f32r)
                for r in range(NCH):
                    nc.sync.dma_start(
                        out=rep1[D1 * r:D1 * (r + 1)], in_=xT[0:D1]
                    )

                zc = []
                for c in range(NCH):
                    rp = psr.tile([KC, TS * 128], f32, name="rp", tag="rp")
                    nc.tensor.matmul(
                        rp, s2_sb[:, c, :], xT[:].rearrange("k s p -> k (s p)"),
                        start=True, stop=True,
                    )
                    z = zp.tile([KC, TS, 128], f32r, name=f"z{c}")
                    nc.vector.tensor_mul(
                        z[:].rearrange("k s p -> k (s p)"),
                        rep1[:].rearrange("k s p -> k (s p)"),
                        rp,
                    )
                    zc.append(z)

                ob = obs.tile([128, TS, DO], f32)
                for s in range(TS):
                    po = pso.tile([128, DO], f32)
                    for c in range(NCH):
                        nc.tensor.matmul(
                            po, zc[c][:, s, :], w_sb[:, c, :],
                            start=(c == 0), stop=(c == NCH - 1),
                        )
                    if s % 2 == 0:
                        nc.vector.tensor_copy(out=ob[:, s, :], in_=po)
                    else:
                        nc.scalar.copy(out=ob[:, s, :], in_=po)
                nc.sync.dma_start(out=outr[t], in_=ob)

    nc.finalize()
    return nc


def _tables(in1_idx, in2_idx, out_idx, cb):
    w2 = np.zeros((DZ, DO), np.float32)
    np.add.at(
        w2,
        (in2_idx.astype(np.int64) * D1 + in1_idx.astype(np.int64),
         out_idx.astype(np.int64)),
        cb.astype(np.float32),
    )
    w2 = w2.reshape(NCH, KC, DO)
    s2 = np.zeros((NCH, DF, KC), np.float32)
    for c in range(NCH):
        for q in range(KC):
            s2[c, D1 + 3 * c + q // D1, q] = 1.0
    return w2, s2


def _get_nc():
    if "nc" not in _CACHE:
        _CACHE["nc"] = _build_program()
    return _CACHE["nc"]


def run_cores(in1, in2, w2, s2, trace=False):
    """Run the SPMD program on 8 cores; returns (out [B, DO], results obj)."""
    from concourse.bass_utils import run_bass_kernel_spmd

    nc = _get_nc()
    in_maps = []
    for c in range(N_CORES):
        in_maps.append({
            "in1": np.ascontiguousarray(in1[c * BC:(c + 1) * BC]),
            "in2": np.ascontiguousarray(in2[c * BC:(c + 1) * BC]),
            "w2": w2,
            "s2": s2,
        })
    res = run_bass_kernel_spmd(
        nc, in_maps, core_ids=list(range(N_CORES)), trace=trace
    )
    out = np.concatenate([res.results[c]["out"] for c in range(N_CORES)], axis=0)
    return out, res


def kernel(in1, in2, in1_idx, in2_idx, out_idx, cb, out_dim):
    in1 = np.asarray(in1, np.float32)
    in2 = np.asarray(in2, np.float32)
    w2, s2 = _tables(
        np.asarray(in1_idx), np.asarray(in2_idx),
        np.asarray(out_idx), np.asarray(cb),
    )
    out, _ = run_cores(in1, in2, w2, s2, trace=False)
    return out.astype(np.float32)

